# revision 14
# baseline (speedup 1.0000x reference)
"""Trainium (trn2) kernel for CurvedRoIExtractor (nn_CurvedRoIExtractor_28295244546862).

kernel(**inputs) takes the FULL inputs (as produced by setup_inputs()) and
returns the FULL output [2, 256, 256, 3, 16] f32.

Sharding: 8 cores = (batch b in {0,1}) x (64-roi quarter).  Features for the
core's batch are passed pre-transposed (channel-last, levels concatenated) in
fp16.  For every sample point and level the two bilinear x-neighbors lie in
consecutive table rows, so one 1KB dma_gather descriptor fetches the (x0,y)
and (x1,y) pixel rows together (elem_size=512 fp16 elems, elem_step=256).
Indices and bilinear weights are precomputed on the host; the device only
gathers, builds masked block-diagonal lhsT tiles once, and runs fp16 matmuls
accumulated over (level, y-neighbor, x-neighbor) in PSUM.  Output is written
fp16 and widened on the host.

Token order (per core): point p = h*1024 + w*64 + roi' (h<3, w<16, roi'<64),
group g = p//32, j = p%32, block B = g//2 (two groups per 128-token block),
u = g%2, y-neighbor n in {0,1}: token t = 128*B + 64*u + 32*n + j.
Gather writes token t to partition t%128, block t//128.  Per (chunk of 4
blocks, level) one dma_gather fetches 512 tokens.  The weighted sum runs as
matmuls: lhsT[q=64u+32n+j, j'=32u+j] = xw_h * yw_n (masked block-diagonal),
rhs = gathered [128, 256] x-half slice, accumulating 8 matmuls (4 levels x
2 x-halves) per 64-point block into PSUM [128, 256] (two blocks per tile via
tile_position).
"""

from contextlib import ExitStack

import numpy as np

import concourse.bass as bass
import concourse.mybir as mybir
import concourse.tile as tile
from concourse import library_config
from concourse.bass_utils import run_bass_kernel_spmd
from concourse.tile import add_dep_helper

F32 = mybir.dt.float32
F16 = mybir.dt.float16
I16 = mybir.dt.int16
AOP = mybir.AluOpType

# (W, H, base row) of each feature level inside the concatenated table
LEVELS = [
    (160, 160, 0),
    (80, 80, 25600),
    (40, 40, 32000),
    (20, 20, 33600),
]
ROWS = 34000
C = 256               # channels
BS = 2
NROI_TOTAL = 256
WP = 16
OUT_H = 3
NPTS = 3072           # per core: 64 rois * 3 * 16
NGRP = NPTS // 32     # 96 groups of 32 points
NBLK = NGRP // 2      # 48 blocks of 2 groups (128 gather tokens each)
NCH = NBLK // 4       # 12 chunks of 4 blocks (512 tokens per level)
NLVL = 4
ICOLS = 2 * NPTS // 16  # idx cols per level (2 y-neighbors per point)


def _fix_waits(nc, max_waits=1):
    """The walrus build in this env rejects >1 sem wait per instruction;
    spill extras onto preceding NOPs on the same engine."""
    for func in nc.m.functions:
        for bb in func.blocks:
            insts = bb.instructions
            for ins in list(insts):
                si = ins.sync_info
                if si is None:
                    continue
                w = list(si.on_wait)
                if len(w) > max_waits:
                    si.on_wait = w[:max_waits]
                    pos = insts.index(ins)
                    extra = w[max_waits:]
                    for k in range(0, len(extra), max_waits):
                        nop = mybir.InstNoOp(
                            name=f"{ins.name}-wf{k}",
                            engine=ins.engine,
                            bass_nofuse=True,
                            sync_info=mybir.SyncInfo(
                                on_wait=extra[k : k + max_waits], on_update=[]
                            ),
                        )
                        insts.insert(pos, nop)
                        pos += 1


def _build_kernel(fix=True):
    nc = bass.Bass("TRN2", target_bir_lowering=False, num_devices=8,
                   num_swdge_queues=4)
    tf = nc.dram_tensor("tfeats", [ROWS, C], F16, kind="ExternalInput")
    # combined prep table: 4 idx tables | wtab (f16 bits) | mask (f16 bits)
    PCOLS = NLVL * ICOLS + 2 * NLVL * NBLK + 64
    prepd = nc.dram_tensor("ptab", [128, PCOLS], I16, kind="ExternalInput")
    outd = nc.dram_tensor("out", [NPTS, C], F16, kind="ExternalOutput")

    with tile.TileContext(nc) as tc, ExitStack() as ctx:
        prep = ctx.enter_context(tc.tile_pool(name="prep", bufs=1))
        gpool = ctx.enter_context(tc.tile_pool(name="g", bufs=5))
        spool = ctx.enter_context(tc.tile_pool(name="stage", bufs=3))
        ppool = ctx.enter_context(tc.tile_pool(name="ps", bufs=7, space="PSUM"))

        nc.gpsimd.load_library(library_config.attnmlp)

        # overlapping 2-row pair view of the table per level:
        # row r -> 512 fp16 elems covering pixel rows r and r+1
        pviews = []
        for (W, H, base) in LEVELS:
            v = tf[base : base + W * H]
            pviews.append(bass.AP(tensor=v.tensor, offset=v.offset,
                                  ap=[[C, W * H - 1], [1, 2 * C]]))

        nreg = nc.gpsimd.to_reg(512)  # shared across all gathers

        pt = prep.tile([128, NLVL * ICOLS + 2 * NLVL * NBLK + 64], I16,
                       tag="pt")
        nc.sync.dma_start(pt[:], prepd[:])
        idxs = [pt[:, l * ICOLS : (l + 1) * ICOLS] for l in range(NLVL)]
        wtab = pt[:, NLVL * ICOLS : NLVL * ICOLS + 2 * NLVL * NBLK].bitcast(F16)
        mask = pt[:, NLVL * ICOLS + 2 * NLVL * NBLK :
                  NLVL * ICOLS + 2 * NLVL * NBLK + 64].bitcast(F16)

        # PE warmup: HAM throttles the PE after idle gaps >3.4us; without
        # this the first ~30us of idle leaves every matmul burst cold.
        # Bridge until the first data-gated matmul (~30us) with dummy
        # matmuls on a zeroed tile into a dedicated full-bank PSUM tile.
        wsrc = prep.tile([128, 512], F16, tag="wsrc")
        nc.vector.memset(wsrc[:], 0)
        wps = ppool.tile([128, 512], F32, tag="warm", bufs=1)  # 1 full bank
        prev_mm = None
        for i in range(100):
            mm = nc.tensor.matmul(
                wps[:],
                wsrc[:, 0:128],
                wsrc[:],
                start=(i == 0),
                stop=(i == 99),
            )
            if prev_mm is not None:
                add_dep_helper(mm.ins, prev_mm.ins, sync=False)
            prev_mm = mm

        # lhsT tables, one per x-half: [128, 4*48 blocks * 64] fp16
        lts = []
        for h in range(2):
            lt = prep.tile([128, NLVL * NBLK * 64], F16, tag=f"lt{h}")
            nc.vector.tensor_tensor(
                lt[:].rearrange("p (b k) -> p b k", k=64),
                mask.unsqueeze(1).to_broadcast([128, NLVL * NBLK, 64]),
                wtab[:, NLVL * NBLK * h : NLVL * NBLK * (h + 1)]
                .unsqueeze(2).to_broadcast([128, NLVL * NBLK, 64]),
                AOP.mult,
            )
            lts.append(lt)

        for ch in range(NCH):
            gts = []
            for l in range(NLVL):
                gt = gpool.tile([128, 4, 2 * C], F16, tag=f"g{l}")
                nc.gpsimd.dma_gather(
                    out_ap=gt[:, 0:4, :],
                    in_ap=pviews[l],
                    idxs_ap=idxs[l][:, ch * 32 : ch * 32 + 32],
                    num_idxs=512,
                    num_idxs_reg=nreg,
                    elem_size=2 * C,
                    elem_step=C,
                    queue_num=l,
                )
                gts.append(gt)
            st = spool.tile([128, 2 * C], F16, tag="st")
            for t in range(2):
                ps = ppool.tile([128, C], F32, tag="ps")
                for u in range(2):
                    blk = 2 * t + u          # block local to chunk
                    B = ch * 4 + blk         # global block
                    nmm = 0
                    for l in range(NLVL):
                        for h in range(2):
                            mm = nc.tensor.matmul(
                                ps[64 * u : 64 * u + 64, :],
                                lts[h][:, (l * NBLK + B) * 64 :
                                        (l * NBLK + B) * 64 + 64],
                                gts[l][:, blk, C * h : C * (h + 1)],
                                start=(nmm == 0),
                                stop=(nmm == 2 * NLVL - 1),
                                tile_position=(0, 64 * u),
                            )
                            nmm += 1
                            # Force PE order: accumulation chains sharing a
                            # PSUM bank must not interleave (start=True
                            # clears the whole bank's has_written bits).
                            if prev_mm is not None:
                                add_dep_helper(mm.ins, prev_mm.ins, sync=False)
                            prev_mm = mm
                nc.vector.tensor_copy(out=st[:, C * t : C * (t + 1)], in_=ps[:])
            r0 = ch * 256
            dview = outd[r0 : r0 + 256, :].rearrange("(t p) c -> p t c", t=2)
            nc.sync.dma_start(dview, st[:].rearrange("p (t c) -> p t c", t=2))

    mybir.codegen_inst_isa_subclasses(nc)
    if fix:
        _fix_waits(nc)
    return nc


# ---------------------------------------------------------------------------
# Host-side prep

def _host_prep_points(center_b, boundary_b, roi0, nroi):
    """Returns (idx tables [4][128, ICOLS] i16, wtab [128, 2*4*NBLK] f16)."""
    bp = boundary_b[roi0 : roi0 + nroi]      # [nroi, Wp, 4]
    cp = center_b[roi0 : roi0 + nroi]        # [nroi, Wp, 2]
    sp = np.stack([bp[..., 0:2], cp, bp[..., 2:4]], axis=1)  # [nroi,3,Wp,2]
    # point order p = h*1024 + w*64 + roi'
    gx = np.ascontiguousarray(sp[..., 0].transpose(1, 2, 0)).reshape(-1)
    gy = np.ascontiguousarray(sp[..., 1].transpose(1, 2, 0)).reshape(-1)
    gx = gx.astype(np.float32)
    gy = gy.astype(np.float32)
    npts = gx.size
    p = np.arange(npts)
    j = p % 32
    u = (p // 32) % 2
    Bg = p // 64
    tok_base = 128 * Bg + 64 * u + j          # token for n=0; n=1 adds 32

    idx_tables = []
    wtab = np.zeros((128, 2 * NLVL * NBLK), np.float32)
    for l, (W, H, base) in enumerate(LEVELS):
        x = ((gx + np.float32(1.0)) * np.float32(0.5)) * np.float32(W - 1)
        y = ((gy + np.float32(1.0)) * np.float32(0.5)) * np.float32(H - 1)
        x0 = np.floor(x)
        y0 = np.floor(y)
        wx = x - x0
        wy = y - y0
        i0 = (y0 * W + x0).astype(np.int32)
        idx_flat = np.zeros(2 * npts, np.int32)
        idx_flat[tok_base] = i0
        idx_flat[tok_base + 32] = i0 + W
        assert idx_flat.max() < 32768
        it = idx_flat.astype(np.int16).reshape(2 * npts // 16, 16).T
        idx_tables.append(
            np.ascontiguousarray(np.tile(it, (8, 1))))
        xw = (np.float32(1.0) - wx, wx)
        yw = (np.float32(1.0) - wy, wy)
        for h in range(2):
            for n in range(2):
                v = (xw[h] * yw[n]).reshape(NBLK, 2, 32)   # [B, u, j]
                for uu in range(2):
                    wtab[64 * uu + 32 * n : 64 * uu + 32 * n + 32,
                         (h * NLVL + l) * NBLK : (h * NLVL + l + 1) * NBLK] = \
                        v[:, uu, :].T
    return idx_tables, np.ascontiguousarray(wtab.astype(np.float16))


def _host_mask():
    q = np.arange(128)[:, None]
    jj = np.arange(64)[None, :]
    return ((q % 32 == jj % 32) & (q // 64 == jj // 32)).astype(np.float16)


def _host_tfeats(feats_b_list, rows=ROWS):
    parts = [np.ascontiguousarray(f.reshape(f.shape[0], -1).T.astype(np.float16))
             for f in feats_b_list]
    tfx = np.concatenate(parts, axis=0)
    assert tfx.shape[0] == rows
    return np.ascontiguousarray(tfx)


_CACHE = {}


def _get_nc():
    if "nc" not in _CACHE:
        _CACHE["nc"] = _build_kernel()
    return _CACHE["nc"]


def kernel(feats0, feats1, feats2, feats3, center_points, boundary_points,
           _want_trace=False, _trace_dir=None):
    feats0 = np.asarray(feats0, dtype=np.float32)
    feats1 = np.asarray(feats1, dtype=np.float32)
    feats2 = np.asarray(feats2, dtype=np.float32)
    feats3 = np.asarray(feats3, dtype=np.float32)
    center_points = np.asarray(center_points, dtype=np.float32)
    boundary_points = np.asarray(boundary_points, dtype=np.float32)

    nc = _get_nc()
    mask = _host_mask()
    tfeats = [
        _host_tfeats([feats0[b], feats1[b], feats2[b], feats3[b]])
        for b in range(BS)
    ]
    nroi = NROI_TOTAL // 4  # 64 rois per core
    in_maps = []
    for core in range(8):
        b = core // 4
        roi0 = (core % 4) * nroi
        idx_tables, wtab = _host_prep_points(
            center_points[b], boundary_points[b], roi0, nroi)
        ptab = np.ascontiguousarray(np.concatenate(
            idx_tables + [wtab.view(np.int16), mask.view(np.int16)], axis=1))
        in_maps.append({"tfeats": tfeats[b], "ptab": ptab})

    kwargs = {}
    if _want_trace:
        kwargs = {"trace": True}
        if _trace_dir is not None:
            kwargs["tmpdir"] = _trace_dir
    res = run_bass_kernel_spmd(nc, in_maps, core_ids=list(range(8)), **kwargs)

    out = np.empty((BS, NROI_TOTAL, C, OUT_H, WP), np.float32)
    for core in range(8):
        b = core // 4
        roi0 = (core % 4) * nroi
        dev = res.results[core]["out"]          # [NPTS, C] f16, rows (h, w, roi')
        o = dev.astype(np.float32).reshape(OUT_H, WP, nroi, C).transpose(2, 3, 0, 1)
        out[b, roi0 : roi0 + nroi] = o
    if _want_trace:
        return out, res
    return out


# revision 18
# speedup vs baseline: 1.1186x; 1.1186x over previous
"""Trainium (trn2) kernel for CurvedRoIExtractor (nn_CurvedRoIExtractor_28295244546862).

kernel(**inputs) takes the FULL inputs (as produced by setup_inputs()) and
returns the FULL output [2, 256, 256, 3, 16] f32.

Sharding: 8 cores = (batch b in {0,1}) x (64-roi quarter).  Features for the
core's batch are passed pre-transposed (channel-last, levels concatenated) in
fp16.  For every sample point and level the two bilinear x-neighbors lie in
consecutive table rows, so one 1KB dma_gather descriptor fetches the (x0,y)
and (x1,y) pixel rows together (elem_size=512 fp16 elems, elem_step=256).
Indices and bilinear weights are precomputed on the host; the device only
gathers, builds masked block-diagonal lhsT tiles once, and runs fp16 matmuls
accumulated over (level, y-neighbor, x-neighbor) in PSUM.  Output is written
fp16 and widened on the host.

Token order (per core): point p = h*1024 + w*64 + roi' (h<3, w<16, roi'<64),
group g = p//32, j = p%32, block B = g//2 (two groups per 128-token block),
u = g%2, y-neighbor n in {0,1}: token t = 128*B + 64*u + 32*n + j.
Gather writes token t to partition t%128, block t//128.  Per (chunk of 4
blocks, level) one dma_gather fetches 512 tokens.  The weighted sum runs as
matmuls: lhsT[q=64u+32n+j, j'=32u+j] = xw_h * yw_n (masked block-diagonal),
rhs = gathered [128, 256] x-half slice, accumulating 8 matmuls (4 levels x
2 x-halves) per 64-point block into PSUM [128, 256] (two blocks per tile via
tile_position).
"""

from contextlib import ExitStack

import numpy as np

import concourse.bass as bass
import concourse.mybir as mybir
import concourse.tile as tile
from concourse import library_config
from concourse.bass_utils import run_bass_kernel_spmd
from concourse.tile import add_dep_helper

F32 = mybir.dt.float32
F16 = mybir.dt.float16
I16 = mybir.dt.int16
AOP = mybir.AluOpType

# (W, H, base row) of each feature level inside the concatenated table
LEVELS = [
    (160, 160, 0),
    (80, 80, 25600),
    (40, 40, 32000),
    (20, 20, 33600),
]
ROWS = 34000
C = 256               # channels
BS = 2
NROI_TOTAL = 256
WP = 16
OUT_H = 3
NPTS = 3072           # per core: 64 rois * 3 * 16
NGRP = NPTS // 32     # 96 groups of 32 points
NBLK = NGRP // 2      # 48 blocks of 2 groups (128 gather tokens each)
NCH = NBLK // 4       # 12 chunks of 4 blocks (512 tokens per level)
NLVL = 4
ICOLS = 2 * NPTS // 16  # idx cols per level (2 y-neighbors per point)


def _fix_waits(nc, max_waits=1):
    """The walrus build in this env rejects >1 sem wait per instruction;
    spill extras onto preceding NOPs on the same engine."""
    for func in nc.m.functions:
        for bb in func.blocks:
            insts = bb.instructions
            for ins in list(insts):
                si = ins.sync_info
                if si is None:
                    continue
                w = list(si.on_wait)
                if len(w) > max_waits:
                    si.on_wait = w[:max_waits]
                    pos = insts.index(ins)
                    extra = w[max_waits:]
                    for k in range(0, len(extra), max_waits):
                        nop = mybir.InstNoOp(
                            name=f"{ins.name}-wf{k}",
                            engine=ins.engine,
                            bass_nofuse=True,
                            sync_info=mybir.SyncInfo(
                                on_wait=extra[k : k + max_waits], on_update=[]
                            ),
                        )
                        insts.insert(pos, nop)
                        pos += 1


def _build_kernel(fix=True):
    nc = bass.Bass("TRN2", target_bir_lowering=False, num_devices=8,
                   num_swdge_queues=4)
    tf = nc.dram_tensor("tfeats", [ROWS, C], F16, kind="ExternalInput")
    # combined prep table: 4 idx tables | wtab (f16 bits) | mask (f16 bits)
    PCOLS = NLVL * ICOLS + 2 * NLVL * NBLK + 64
    prepd = nc.dram_tensor("ptab", [128, PCOLS], I16, kind="ExternalInput")
    outd = nc.dram_tensor("out", [NPTS, C], F16, kind="ExternalOutput")

    with tile.TileContext(nc) as tc, ExitStack() as ctx:
        prep = ctx.enter_context(tc.tile_pool(name="prep", bufs=1))
        gpool = ctx.enter_context(tc.tile_pool(name="g", bufs=5))
        spool = ctx.enter_context(tc.tile_pool(name="stage", bufs=3))
        ppool = ctx.enter_context(tc.tile_pool(name="ps", bufs=8, space="PSUM"))

        nc.gpsimd.load_library(library_config.attnmlp)

        # overlapping 2-row pair view of the table per level:
        # row r -> 512 fp16 elems covering pixel rows r and r+1
        pviews = []
        for (W, H, base) in LEVELS:
            v = tf[base : base + W * H]
            pviews.append(bass.AP(tensor=v.tensor, offset=v.offset,
                                  ap=[[C, W * H - 1], [1, 2 * C]]))

        nreg = nc.gpsimd.to_reg(512)  # shared across all gathers

        pt = prep.tile([128, NLVL * ICOLS + 2 * NLVL * NBLK + 64], I16,
                       tag="pt")
        nc.sync.dma_start(pt[:], prepd[:])
        idxs = [pt[:, l * ICOLS : (l + 1) * ICOLS] for l in range(NLVL)]
        wtab = pt[:, NLVL * ICOLS : NLVL * ICOLS + 2 * NLVL * NBLK].bitcast(F16)
        mask = pt[:, NLVL * ICOLS + 2 * NLVL * NBLK :
                  NLVL * ICOLS + 2 * NLVL * NBLK + 64].bitcast(F16)

        # lhsT tables, one per x-half: [128, 4*48 blocks * 64] fp16
        lts = []
        for h in range(2):
            lt = prep.tile([128, NLVL * NBLK * 64], F16, tag=f"lt{h}")
            nc.vector.tensor_tensor(
                lt[:].rearrange("p (b k) -> p b k", k=64),
                mask.unsqueeze(1).to_broadcast([128, NLVL * NBLK, 64]),
                wtab[:, NLVL * NBLK * h : NLVL * NBLK * (h + 1)]
                .unsqueeze(2).to_broadcast([128, NLVL * NBLK, 64]),
                AOP.mult,
            )
            lts.append(lt)

        prev_mm = None
        for ch in range(NCH):
            gts = []
            for l in range(NLVL):
                gt = gpool.tile([128, 4, 2 * C], F16, tag=f"g{l}")
                nc.gpsimd.dma_gather(
                    out_ap=gt[:, 0:4, :],
                    in_ap=pviews[l],
                    idxs_ap=idxs[l][:, ch * 32 : ch * 32 + 32],
                    num_idxs=512,
                    num_idxs_reg=nreg,
                    elem_size=2 * C,
                    elem_step=C,
                    queue_num=l,
                )
                gts.append(gt)
            st = spool.tile([128, 2 * C], F16, tag="st")
            # level-outer matmul order: each level's matmuls run as soon as
            # that level's gather lands, so the chunk's compute overlaps its
            # own gathers (shortens the tail after the final gather).  The
            # two PSUM tiles accumulate interleaved — safe because PSUM
            # tiles are bank-granular, so start=True only clears its own
            # tile's bank.  PE order is still forced via the global chain.
            pss = [ppool.tile([128, C], F32, tag="ps", name=f"ps{ch}_{t}")
                   for t in range(2)]
            for l in range(NLVL):
                for t in range(2):
                    for u in range(2):
                        blk = 2 * t + u          # block local to chunk
                        B = ch * 4 + blk         # global block
                        for h in range(2):
                            mm = nc.tensor.matmul(
                                pss[t][64 * u : 64 * u + 64, :],
                                lts[h][:, (l * NBLK + B) * 64 :
                                        (l * NBLK + B) * 64 + 64],
                                gts[l][:, blk, C * h : C * (h + 1)],
                                start=(l == 0 and h == 0),
                                stop=(l == NLVL - 1 and h == 1),
                                tile_position=(0, 64 * u),
                            )
                            if prev_mm is not None:
                                add_dep_helper(mm.ins, prev_mm.ins, sync=False)
                            prev_mm = mm
            for t in range(2):
                nc.vector.tensor_copy(out=st[:, C * t : C * (t + 1)],
                                      in_=pss[t][:])
            r0 = ch * 256
            dview = outd[r0 : r0 + 256, :].rearrange("(t p) c -> p t c", t=2)
            nc.sync.dma_start(dview, st[:].rearrange("p (t c) -> p t c", t=2))

    mybir.codegen_inst_isa_subclasses(nc)
    if fix:
        _fix_waits(nc)
    return nc


# ---------------------------------------------------------------------------
# Host-side prep

def _host_prep_points(center_b, boundary_b, roi0, nroi):
    """Returns (idx tables [4][128, ICOLS] i16, wtab [128, 2*4*NBLK] f16)."""
    bp = boundary_b[roi0 : roi0 + nroi]      # [nroi, Wp, 4]
    cp = center_b[roi0 : roi0 + nroi]        # [nroi, Wp, 2]
    sp = np.stack([bp[..., 0:2], cp, bp[..., 2:4]], axis=1)  # [nroi,3,Wp,2]
    # point order p = h*1024 + w*64 + roi'
    gx = np.ascontiguousarray(sp[..., 0].transpose(1, 2, 0)).reshape(-1)
    gy = np.ascontiguousarray(sp[..., 1].transpose(1, 2, 0)).reshape(-1)
    gx = gx.astype(np.float32)
    gy = gy.astype(np.float32)
    npts = gx.size
    p = np.arange(npts)
    j = p % 32
    u = (p // 32) % 2
    Bg = p // 64
    tok_base = 128 * Bg + 64 * u + j          # token for n=0; n=1 adds 32

    idx_tables = []
    wtab = np.zeros((128, 2 * NLVL * NBLK), np.float32)
    for l, (W, H, base) in enumerate(LEVELS):
        x = ((gx + np.float32(1.0)) * np.float32(0.5)) * np.float32(W - 1)
        y = ((gy + np.float32(1.0)) * np.float32(0.5)) * np.float32(H - 1)
        x0 = np.floor(x)
        y0 = np.floor(y)
        wx = x - x0
        wy = y - y0
        i0 = (y0 * W + x0).astype(np.int32)
        idx_flat = np.zeros(2 * npts, np.int32)
        idx_flat[tok_base] = i0
        idx_flat[tok_base + 32] = i0 + W
        assert idx_flat.max() < 32768
        it = idx_flat.astype(np.int16).reshape(2 * npts // 16, 16).T
        idx_tables.append(
            np.ascontiguousarray(np.tile(it, (8, 1))))
        xw = (np.float32(1.0) - wx, wx)
        yw = (np.float32(1.0) - wy, wy)
        for h in range(2):
            for n in range(2):
                v = (xw[h] * yw[n]).reshape(NBLK, 2, 32)   # [B, u, j]
                for uu in range(2):
                    wtab[64 * uu + 32 * n : 64 * uu + 32 * n + 32,
                         (h * NLVL + l) * NBLK : (h * NLVL + l + 1) * NBLK] = \
                        v[:, uu, :].T
    return idx_tables, np.ascontiguousarray(wtab.astype(np.float16))


def _host_mask():
    q = np.arange(128)[:, None]
    jj = np.arange(64)[None, :]
    return ((q % 32 == jj % 32) & (q // 64 == jj // 32)).astype(np.float16)


def _host_tfeats(feats_b_list, rows=ROWS):
    parts = [np.ascontiguousarray(f.reshape(f.shape[0], -1).T.astype(np.float16))
             for f in feats_b_list]
    tfx = np.concatenate(parts, axis=0)
    assert tfx.shape[0] == rows
    return np.ascontiguousarray(tfx)


_CACHE = {}


def _get_nc():
    if "nc" not in _CACHE:
        _CACHE["nc"] = _build_kernel()
    return _CACHE["nc"]


def kernel(feats0, feats1, feats2, feats3, center_points, boundary_points,
           _want_trace=False, _trace_dir=None):
    feats0 = np.asarray(feats0, dtype=np.float32)
    feats1 = np.asarray(feats1, dtype=np.float32)
    feats2 = np.asarray(feats2, dtype=np.float32)
    feats3 = np.asarray(feats3, dtype=np.float32)
    center_points = np.asarray(center_points, dtype=np.float32)
    boundary_points = np.asarray(boundary_points, dtype=np.float32)

    nc = _get_nc()
    mask = _host_mask()
    tfeats = [
        _host_tfeats([feats0[b], feats1[b], feats2[b], feats3[b]])
        for b in range(BS)
    ]
    nroi = NROI_TOTAL // 4  # 64 rois per core
    in_maps = []
    for core in range(8):
        b = core // 4
        roi0 = (core % 4) * nroi
        idx_tables, wtab = _host_prep_points(
            center_points[b], boundary_points[b], roi0, nroi)
        ptab = np.ascontiguousarray(np.concatenate(
            idx_tables + [wtab.view(np.int16), mask.view(np.int16)], axis=1))
        in_maps.append({"tfeats": tfeats[b], "ptab": ptab})

    kwargs = {}
    if _want_trace:
        kwargs = {"trace": True}
        if _trace_dir is not None:
            kwargs["tmpdir"] = _trace_dir
    res = run_bass_kernel_spmd(nc, in_maps, core_ids=list(range(8)), **kwargs)

    out = np.empty((BS, NROI_TOTAL, C, OUT_H, WP), np.float32)
    for core in range(8):
        b = core // 4
        roi0 = (core % 4) * nroi
        dev = res.results[core]["out"]          # [NPTS, C] f16, rows (h, w, roi')
        o = dev.astype(np.float32).reshape(OUT_H, WP, nroi, C).transpose(2, 3, 0, 1)
        out[b, roi0 : roi0 + nroi] = o
    if _want_trace:
        return out, res
    return out
